# revision 21
# baseline (speedup 1.0000x reference)
"""DeepSeek MLA prefill (absorbed) on 8 Trainium2 NeuronCores — v4.

Sharding: tensor-parallel over heads (2 of 16 per core). Host-side
algebraic folds remove both the QKV-compression collective and most of
the attention FLOPs:
  - W_comb = W_cqkv[:, qc] @ W_uq   -> q computed straight from x.
  - k_abs = kv_c @ W_qk[h].T on device (192-dim score contraction).
  - VW[h] = V @ W_o1[h] on host     -> value+O-bmm in one matmul.
Top-k selection = dense count-weighted softmax. All f16, f32 PSUM.

Phases: q-GEMM streams x/W_comb first (overlapping the collectives'
init window), then k_abs, then attention twice over 256-token halves
so half A's o2 AllGather rides under half B's compute and the
O-projection pipelines under half B's AllGather. Inside a half,
sc-chunks run in pairs three-deep pipelined: scores (PE) -> exp (ACT,
one op per pair) -> xcnt (DVE) -> Z + value (PE). Z for both heads
packs into one PSUM bank (single start per bank; has_written handles
the second group).
"""

import os
import sys

sys.path.insert(0, "/opt/trn_rl_repo")

import numpy as np

import concourse.bass as bass
import concourse.tile as tile
from concourse import bacc, mybir
from concourse.bass_utils import run_bass_kernel_spmd

F32 = mybir.dt.float32
F16 = mybir.dt.float16

N_CORES = 8
M = 512
MH = M // 2
HID = 7168
D_KV_C, D_Q_C, D_R, D_Q = 512, 1536, 64, 128
H_LOC = 2
D_ATT = D_KV_C + D_R
S_KV = 4096
OUT_C = HID // N_CORES          # 896
KH = HID // 128                 # 56
NSC = S_KV // 128               # 32
NPR = NSC // 2                  # 16 sc-pairs
SM_SCALE = 1.0 / float(np.sqrt(np.float32(D_ATT)))
N_WARM = 48
LAG = 2


def build_program():
    nc = bacc.Bacc("TRN2", target_bir_lowering=False, debug=False,
                   num_devices=N_CORES)

    xp = nc.dram_tensor("xp", [128, KH * M], F16, kind="ExternalInput")
    wcp = nc.dram_tensor("wcp", [128, KH * 384], F16, kind="ExternalInput")
    wqk2 = nc.dram_tensor("wqk2", [128, H_LOC * 4 * 128], F16,
                          kind="ExternalInput")
    kvp = nc.dram_tensor("kvp", [128, 4 * S_KV], F16, kind="ExternalInput")
    kpe2d = nc.dram_tensor("kpe2d", [128, S_KV], F16, kind="ExternalInput")
    cntp = nc.dram_tensor("cntp", [128, 2 * NPR * M], F16,
                          kind="ExternalInput")
    vwp = nc.dram_tensor("vwp", [128, H_LOC * NSC * 128], F16,
                         kind="ExternalInput")
    wopp = nc.dram_tensor("wopp", [128, 16 * OUT_C], F16,
                          kind="ExternalInput")
    outT = nc.dram_tensor("outT", [OUT_C, M], F32, kind="ExternalOutput")

    rg = [list(range(N_CORES))]

    with tile.TileContext(nc) as tc, \
            nc.allow_low_precision(reason="f16 matmul pipeline, f32 accum"):
        with tc.tile_pool(name="dram", bufs=1, space="DRAM") as dram:
            o2l = [dram.tile([H_LOC * 128, MH], F16, name=f"o2l{mh}")
                   for mh in range(2)]
            o2a = [dram.tile([H_LOC * 128 * N_CORES, MH], F16,
                             name=f"o2a{mh}", addr_space="Shared")
                   for mh in range(2)]

            per_cm = tc.tile_pool(name="per", bufs=1)
            per = per_cm.__enter__()
            kabs = [per.tile([128, S_KV], F16, name=f"kabs{h}")
                    for h in range(H_LOC)]
            kpe_sb = per.tile([128, S_KV], F16, name="kpe")
            qn = [per.tile([128, M], F16, name=f"qn{h}")
                  for h in range(H_LOC)]
            qpe2 = per.tile([128, M], F16, name="qpe2")
            vw_sb = per.tile([128, H_LOC * NSC * 128], F16, name="vw")
            wop_sb = per.tile([128, 16 * OUT_C], F16, name="wop")
            wc_sb = per.tile([128, KH * 384], F16, name="wc")
            xbd = [per.tile([128, 2048], F16, name=f"xbd{g}")
                   for g in range(KH // 8)]
            warm = per.tile([128, 64], F32, name="warm")
            ones_col = per.tile([128, 1], F16, name="ones_col")
            ones_row = per.tile([1, 128], F32, name="ones_row")
            nc.vector.memset(warm[:], 0.0)
            nc.vector.memset(ones_col[:], 1.0)
            nc.vector.memset(ones_row[:], 1.0)

            def stage12(mh, s2x, psq):
                """q[:, mh-half] = x_half @ W_comb (wc_sb resident)."""
                pq = [psq.tile([128, 512], F32, name=f"q{j}", tag=f"q{j}")
                      for j in range(3)]
                NG = KH // 8
                for g in range(NG):
                    xa = s2x.tile([128, 2048], F16, name="xa", tag="xa")
                    nc.sync.dma_start(
                        xa[:], xp[:, (mh * KH + 8 * g) * MH:
                                  (mh * KH + 8 * g + 8) * MH])
                    if mh == 0:
                        nc.sync.dma_start(
                            wc_sb[:, g * 8 * 384:(g + 1) * 8 * 384],
                            wcp[:, g * 8 * 384:(g + 1) * 8 * 384])
                        if 1 <= g <= 4:
                            pcs = [(0, 0), (1, 0), (2, 0), (3, 0),
                                   (0, 1), (1, 1), (2, 1), (3, 1)]
                            for c, half in pcs[(g - 1) * 2:(g - 1) * 2 + 2]:
                                nc.sync.dma_start(
                                    kv_sb[:, c * S_KV + half * 2048:
                                          c * S_KV + (half + 1) * 2048],
                                    kvp[:, c * S_KV + half * 2048:
                                        c * S_KV + (half + 1) * 2048])
                        if g == 5:
                            nc.sync.dma_start(wqk_sb[:], wqk2[:])
                            nc.sync.dma_start(kpe_sb[:, 0:2048],
                                              kpe2d[:, 0:2048])
                            nc.sync.dma_start(kpe_sb[:, 2048:4096],
                                              kpe2d[:, 2048:4096])
                    for kk in range(8):
                        rhs = xa[:, kk * MH:(kk + 1) * MH]
                        for j in range(3):
                            nc.tensor.matmul(
                                pq[j][:, 0:MH],
                                wc_sb[:, (g * 8 + kk) * 384 + j * 128:
                                      (g * 8 + kk) * 384 + (j + 1) * 128],
                                rhs,
                                start=(g == 0 and kk == 0),
                                stop=(g == NG - 1 and kk == 7),
                                skip_group_check=True)
                m0 = mh * MH
                nc.vector.tensor_copy(qn[0][:, m0:m0 + MH], pq[0][:, 0:MH])
                nc.scalar.copy(qn[1][:, m0:m0 + MH], pq[1][:, 0:MH])
                nc.vector.tensor_copy(qpe2[:, m0:m0 + MH], pq[2][:, 0:MH])

            s1_cm = tc.tile_pool(name="s1", bufs=1)
            s1 = s1_cm.__enter__()
            kv_sb = s1.tile([128, 4 * S_KV], F16, name="kv")
            wqk_sb = s1.tile([128, H_LOC * 4 * 128], F16, name="wqk")

            # ---------------- warmup + stage12 half A --------------------
            with (
                tc.tile_pool(name="s2xa", bufs=3) as s2x,
                tc.tile_pool(name="psqa", bufs=1, space="PSUM") as psq,
                tc.tile_pool(name="psw", bufs=1, space="PSUM") as psw,
            ):
                wps = psw.tile([1, 64], F32, name="wps", tag="wps")
                for i in range(N_WARM):
                    nc.tensor.matmul(wps[:], warm[:, 0:1], warm[:],
                                     start=(i == 0), stop=(i == N_WARM - 1),
                                     skip_group_check=True)
                stage12(0, s2x, psq)

            # ---------------- k_abs --------------------------------------
            with tc.tile_pool(name="psk", bufs=2, space="PSUM") as psk:
                nc.sync.dma_start(vw_sb[:, 0:S_KV], vwp[:, 0:S_KV])
                nc.sync.dma_start(vw_sb[:, S_KV:2 * S_KV],
                                  vwp[:, S_KV:2 * S_KV])
                for pc in range(8):
                    for h in range(H_LOC):
                        kps = psk.tile([128, 512], F32, name="kps",
                                       tag="kps")
                        for c in range(4):
                            nc.tensor.matmul(
                                kps[:],
                                wqk_sb[:, (h * 4 + c) * 128:
                                       (h * 4 + c + 1) * 128],
                                kv_sb[:, c * S_KV + pc * 512:
                                      c * S_KV + (pc + 1) * 512],
                                start=(c == 0), stop=(c == 3))
                        if h == 0:
                            nc.vector.tensor_copy(
                                kabs[h][:, pc * 512:(pc + 1) * 512], kps[:])
                        else:
                            nc.scalar.copy(
                                kabs[h][:, pc * 512:(pc + 1) * 512], kps[:])
            s1_cm.__exit__(None, None, None)

            def stage12b(psq):
                pq = [psq.tile([128, 512], F32, name=f"qb{j}",
                               tag=f"qb{j}") for j in range(3)]
                NG = KH // 8
                for g in range(NG):
                    for kk in range(8):
                        rhs = xbd[g][:, kk * MH:(kk + 1) * MH]
                        for j in range(3):
                            nc.tensor.matmul(
                                pq[j][:, 0:MH],
                                wc_sb[:, (g * 8 + kk) * 384 + j * 128:
                                      (g * 8 + kk) * 384 + (j + 1) * 128],
                                rhs,
                                start=(g == 0 and kk == 0),
                                stop=(g == NG - 1 and kk == 7),
                                skip_group_check=True)
                nc.vector.tensor_copy(qn[0][:, MH:M], pq[0][:, 0:MH])
                nc.scalar.copy(qn[1][:, MH:M], pq[1][:, 0:MH])
                nc.vector.tensor_copy(qpe2[:, MH:M], pq[2][:, 0:MH])

            # ---------------- attention + deferred work ------------------
            for mh in range(2):
                m0 = mh * MH
                with (
                    tc.tile_pool(name=f"cnt{mh}", bufs=8) as cnts,
                    tc.tile_pool(name=f"exs{mh}", bufs=4) as exs,
                    tc.tile_pool(name=f"pts{mh}", bufs=4) as pts,
                    tc.tile_pool(name=f"psS{mh}", bufs=LAG,
                                 space="PSUM") as psS,
                    tc.tile_pool(name=f"psZ{mh}", bufs=1,
                                 space="PSUM") as psZ,
                ):
                    o2p = psZ.tile([128, 2 * MH], F32, name="o2p")
                    z_ps = [psZ.tile([1, M], F32, name=f"zp{h}")
                            for h in range(H_LOC)]
                    cnt_t = {}
                    # preload the whole half's counts (8 x 256 KB)
                    for pr2 in range(NPR // 2):
                        t = cnts.tile([128, 2 * M], F16, name="cc",
                                      tag="cc")
                        nc.sync.dma_start(
                            t[:], cntp[:, (mh * NPR + 2 * pr2) * M:
                                       (mh * NPR + 2 * pr2 + 2) * M])
                        cnt_t[2 * pr2] = t[:, 0:M]
                        cnt_t[2 * pr2 + 1] = t[:, M:2 * M]
                    if mh == 0:
                        # defer: half B x, O-proj weights (used later)
                        for g in range(KH // 8):
                            nc.sync.dma_start(
                                xbd[g][:], xp[:, (KH + 8 * g) * MH:
                                              (KH + 8 * g + 8) * MH])
                        for w8 in range(8):
                            nc.sync.dma_start(
                                wop_sb[:, w8 * 2 * OUT_C:
                                       (w8 + 1) * 2 * OUT_C],
                                wopp[:, w8 * 2 * OUT_C:
                                     (w8 + 1) * 2 * OUT_C])

                    pend = {}

                    def emit_scores(pr):
                        ps = psS.tile([128, 4 * MH], F32, name="ss",
                                      tag="ss")
                        for q in range(2):
                            sc = 2 * pr + q
                            for h in range(H_LOC):
                                nc.tensor.matmul(
                                    ps[:, (h * 2 + q) * MH:
                                       (h * 2 + q + 1) * MH],
                                    kabs[h][:, sc * 128:(sc + 1) * 128],
                                    qn[h][:, m0:m0 + MH],
                                    start=(q == 0), stop=False,
                                    skip_group_check=True)
                        for q in range(2):
                            sc = 2 * pr + q
                            for h in range(H_LOC):
                                b = h * 64
                                nc.tensor.matmul(
                                    ps[:, (h * 2 + q) * MH:
                                       (h * 2 + q + 1) * MH],
                                    kpe_sb[b:b + 64,
                                           sc * 128:(sc + 1) * 128],
                                    qpe2[b:b + 64, m0:m0 + MH],
                                    start=False, stop=True,
                                    skip_group_check=True,
                                    tile_position=(b, 0))
                        pend[pr] = ps

                    def consume(pr):
                        ps = pend.pop(pr)
                        ex = exs.tile([128, 4 * MH], F16, name="ex",
                                      tag="ex")
                        nc.scalar.activation(
                            ex[:], ps[:],
                            mybir.ActivationFunctionType.Exp,
                            scale=SM_SCALE)
                        for h in range(H_LOC):
                            pt = pts.tile([128, 2 * MH], F16, name="pt",
                                          tag="pt")
                            nc.vector.tensor_mul(
                                pt[:], ex[:, h * 2 * MH:(h + 1) * 2 * MH],
                                cnt_t[pr])
                            nc.tensor.matmul(
                                z_ps[h][:], ones_col[:], pt[:],
                                start=(pr == 0), stop=(pr == NPR - 1),
                                skip_group_check=True)
                            for q in range(2):
                                sc = 2 * pr + q
                                nc.tensor.matmul(
                                    o2p[:, h * MH:(h + 1) * MH],
                                    vw_sb[:, (h * NSC + sc) * 128:
                                          (h * NSC + sc + 1) * 128],
                                    pt[:, q * MH:(q + 1) * MH],
                                    start=(pr == 0 and q == 0 and h == 0),
                                    stop=(pr == NPR - 1 and q == 1),
                                    skip_group_check=True)

                    for pr in range(NPR):
                        emit_scores(pr)
                        if pr >= LAG:
                            consume(pr - LAG)
                    for pr in range(NPR - LAG, NPR):
                        consume(pr)

                    # Z -> 1/Z broadcast -> normalize -> AllGather
                    zs_l = []
                    for h in range(H_LOC):
                        zsb = exs.tile([1, M], F32, name="zsb", tag="zsb")
                        nc.scalar.copy(zsb[:], z_ps[h][:])
                        zs = exs.tile([1, MH], F32, name="zs", tag="zs")
                        nc.vector.tensor_add(zs[:], zsb[0:1, 0:MH],
                                             zsb[0:1, MH:2 * MH])
                        zs_l.append(zs)
                    zbt = psS.tile([128, 4 * MH], F32, name="ss", tag="ss")
                    nc.tensor.matmul(zbt[:, 0:MH], ones_row[:],
                                     zs_l[0][:], start=True, stop=True,
                                     skip_group_check=True)
                    nc.tensor.matmul(zbt[:, MH:2 * MH], ones_row[:],
                                     zs_l[1][:], start=False, stop=True,
                                     skip_group_check=True)
                    zbsb = exs.tile([128, 2 * MH], F32, name="zbsb",
                                    tag="zbsb")
                    nc.scalar.copy(zbsb[:], zbt[:, 0:2 * MH])
                    rzb = exs.tile([128, 2 * MH], F32, name="rzb",
                                   tag="rzb")
                    nc.vector.reciprocal_approx_fast(rzb[:], zbsb[:])
                    for h in range(H_LOC):
                        o2s = pts.tile([128, MH], F16, name=f"o2s{h}")
                        nc.vector.tensor_mul(
                            o2s[:], o2p[:, h * MH:(h + 1) * MH],
                            rzb[:, h * MH:(h + 1) * MH])
                        nc.sync.dma_start(
                            o2l[mh][h * 128:(h + 1) * 128, :], o2s[:])
                    nc.gpsimd.collective_compute(
                        "AllGather", mybir.AluOpType.bypass,
                        replica_groups=rg,
                        ins=[o2l[mh].opt()], outs=[o2a[mh].opt()])
                    for f in range(4):
                        nc.tensor.matmul(
                            zbt[:, 2 * MH:4 * MH],
                            vw_sb[:, 0:128], vw_sb[:, 0:M],
                            start=(f == 0), stop=True,
                            skip_group_check=True)

                if mh == 0:
                    # stage12 for half B rides under half A's AllGather
                    with tc.tile_pool(name="psqb", bufs=1,
                                      space="PSUM") as psqb:
                        stage12b(psqb)

            # ---------------- O-projection (per half, p-outer) -----------
            for mh in range(2):
                with (
                    tc.tile_pool(name=f"s6{mh}", bufs=1) as s6,
                    tc.tile_pool(name=f"ps6{mh}", bufs=2,
                                 space="PSUM") as ps6,
                    tc.tile_pool(name=f"s6o{mh}", bufs=4) as s6o,
                ):
                    o2t = []
                    for n in range(16):
                        ok = s6.tile([128, MH], F16, name=f"o2t{n}")
                        nc.sync.dma_start(
                            ok[:], o2a[mh][n * 128:(n + 1) * 128, :])
                        o2t.append(ok)
                    if mh == 1:
                        fill = ps6.tile([128, 512], F32, name="pb",
                                        tag="pb")
                        for f in range(12):
                            nc.tensor.matmul(
                                fill[:, 0:M], vw_sb[:, 0:128],
                                vw_sb[:, 0:M], start=(f == 0), stop=True,
                                skip_group_check=True)
                    for p in range(OUT_C // 128):
                        pb = ps6.tile([128, 512], F32, name="pb", tag="pb")
                        for n, ok in enumerate(o2t):
                            nc.tensor.matmul(
                                pb[:, 0:MH],
                                wop_sb[:, n * OUT_C + p * 128:
                                       n * OUT_C + (p + 1) * 128],
                                ok[:], start=(n == 0), stop=(n == 15),
                                skip_group_check=True)
                        ob = s6o.tile([128, MH], F32, name="outb",
                                      tag="outb")
                        if p % 2 == 0:
                            nc.vector.tensor_copy(ob[:], pb[:, 0:MH])
                        else:
                            nc.scalar.copy(ob[:], pb[:, 0:MH])
                        nc.sync.dma_start(
                            outT[p * 128:(p + 1) * 128,
                                 mh * MH:(mh + 1) * MH], ob[:])

            per_cm.__exit__(None, None, None)

    nc.compile()
    return nc


def prep_inputs(x, W_cqkv, W_uq, W_qk, kv_cache, W_o1, W_oproj, indices):
    x = np.asarray(x, np.float32)
    W_cqkv = np.asarray(W_cqkv, np.float32)
    W_uq = np.asarray(W_uq, np.float32)
    W_qk = np.asarray(W_qk, np.float32)
    kv_cache = np.asarray(kv_cache, np.float32)
    W_o1 = np.asarray(W_o1, np.float32)
    W_oproj = np.asarray(W_oproj, np.float32)
    indices = np.asarray(indices)

    # host-side algebraic folds (f32)
    w_comb = W_cqkv[:, D_KV_C:D_KV_C + D_Q_C] @ W_uq        # [7168, 3072]
    vw = np.einsum("sc,hcv->hsv", kv_cache[:, :D_KV_C], W_o1)  # [16,4096,128]

    def pack(a, nchunk):
        # [nchunk*128, F] -> [128, nchunk*F]
        f = a.shape[1]
        return np.ascontiguousarray(
            a.reshape(nchunk, 128, f).transpose(1, 0, 2).reshape(
                128, nchunk * f))

    kvT = kv_cache.T                                         # [576, 4096]
    kvp = pack(kvT[:D_KV_C], 4).astype(np.float16)
    kpe2 = np.concatenate([kvT[D_KV_C:], kvT[D_KV_C:]], 0).astype(np.float16)

    cm = np.zeros((M, S_KV), np.float32)
    np.add.at(cm, (np.arange(M)[:, None], indices), 1.0)
    # cnt_pack[p, mh, pr, q, m256] = cm[mh*256+m, 128*(2pr+q)+p]
    cmT = np.ascontiguousarray(cm.T).reshape(NSC, 128, 2, MH)
    cntp = np.empty((128, 2, NPR, 2, MH), np.float32)
    for pr in range(NPR):
        for q in range(2):
            sc = 2 * pr + q
            for mhh in range(2):
                cntp[:, mhh, pr, q, :] = cmT[sc, :, mhh, :]
    cntp = np.ascontiguousarray(
        cntp.reshape(128, 2 * NPR * M)).astype(np.float16)

    # x_pack2[p, mh, k, m256] = x[mh*256+m, 128k+p]
    xT3 = np.ascontiguousarray(x.T).reshape(KH, 128, 2, MH)
    xpk = np.ascontiguousarray(
        xT3.transpose(1, 2, 0, 3).reshape(128, KH * M)).astype(np.float16)

    in_maps = []
    for i in range(N_CORES):
        h0 = i * H_LOC
        c0 = i * OUT_C
        cols = np.concatenate([
            w_comb[:, (h0 + 0) * 192:(h0 + 0) * 192 + 128],
            w_comb[:, (h0 + 1) * 192:(h0 + 1) * 192 + 128],
            w_comb[:, (h0 + 0) * 192 + 128:(h0 + 1) * 192],
            w_comb[:, (h0 + 1) * 192 + 128:(h0 + 2) * 192],
        ], axis=1)                                           # [7168, 384]
        wcpk = pack(cols, KH).astype(np.float16)

        wqk_l = np.stack([
            pack(np.ascontiguousarray(W_qk[h].T), 4)
            for h in range(h0, h0 + H_LOC)], axis=1)
        wqk_l = np.ascontiguousarray(
            wqk_l.reshape(128, H_LOC * 4 * 128)).astype(np.float16)

        vw_l = np.stack([pack(vw[h], NSC)
                         for h in range(h0, h0 + H_LOC)], axis=1)
        vw_l = np.ascontiguousarray(
            vw_l.reshape(128, H_LOC * NSC * 128)).astype(np.float16)

        # gathered chunk n = rank n//2, local head n%2 = global head n
        order = [2 * k + h for k in range(N_CORES) for h in range(H_LOC)]
        wop_r = W_oproj.reshape(16, 128, HID)[order][:, :, c0:c0 + OUT_C]
        wop_l = pack(wop_r.reshape(16 * 128, OUT_C), 16).astype(np.float16)

        in_maps.append({
            "xp": xpk,
            "wcp": wcpk,
            "wqk2": wqk_l,
            "kvp": kvp,
            "kpe2d": kpe2,
            "cntp": cntp,
            "vwp": vw_l,
            "wopp": wop_l,
        })
    return in_maps


_prog_cache = {}


def kernel(x, W_cqkv, W_uq, W_qk, kv_cache, W_o1, W_oproj, indices):
    if "nc" not in _prog_cache:
        _prog_cache["nc"] = build_program()
    nc = _prog_cache["nc"]
    in_maps = prep_inputs(x, W_cqkv, W_uq, W_qk, kv_cache, W_o1, W_oproj,
                          indices)
    trace = bool(int(os.environ.get("KERNEL_TRACE", "0")))
    res = run_bass_kernel_spmd(nc, in_maps, list(range(N_CORES)),
                               trace=trace)
    _prog_cache["last_result"] = res
    out = np.empty((M, HID), np.float32)
    for i in range(N_CORES):
        out[:, i * OUT_C:(i + 1) * OUT_C] = res.results[i]["outT"].T
    return out


# revision 22
# speedup vs baseline: 1.0512x; 1.0512x over previous
"""DeepSeek MLA prefill (absorbed) on 8 Trainium2 NeuronCores — v4.

Sharding: tensor-parallel over heads (2 of 16 per core). Host-side
algebraic folds remove both the QKV-compression collective and most of
the attention FLOPs:
  - W_comb = W_cqkv[:, qc] @ W_uq   -> q computed straight from x.
  - k_abs = kv_c @ W_qk[h].T on device (192-dim score contraction).
  - VW[h] = V @ W_o1[h] on host     -> value+O-bmm in one matmul.
Top-k selection = dense count-weighted softmax. All f16, f32 PSUM.

Phases: q-GEMM streams x/W_comb first (overlapping the collectives'
init window), then k_abs, then attention twice over 256-token halves
so half A's o2 AllGather rides under half B's compute and the
O-projection pipelines under half B's AllGather. Inside a half,
sc-chunks run in pairs three-deep pipelined: scores (PE) -> exp (ACT,
one op per pair) -> xcnt (DVE) -> Z + value (PE). Z for both heads
packs into one PSUM bank (single start per bank; has_written handles
the second group).
"""

import os
import sys

sys.path.insert(0, "/opt/trn_rl_repo")

import numpy as np

import concourse.bass as bass
import concourse.tile as tile
from concourse import bacc, mybir
from concourse.bass_utils import run_bass_kernel_spmd

F32 = mybir.dt.float32
F16 = mybir.dt.float16

N_CORES = 8
M = 512
MH = M // 2
HID = 7168
D_KV_C, D_Q_C, D_R, D_Q = 512, 1536, 64, 128
H_LOC = 2
D_ATT = D_KV_C + D_R
S_KV = 4096
OUT_C = HID // N_CORES          # 896
KH = HID // 128                 # 56
NSC = S_KV // 128               # 32
NPR = NSC // 2                  # 16 sc-pairs
SM_SCALE = 1.0 / float(np.sqrt(np.float32(D_ATT)))
N_WARM = 48
LAG = 2


def build_program():
    nc = bacc.Bacc("TRN2", target_bir_lowering=False, debug=False,
                   num_devices=N_CORES)

    xp = nc.dram_tensor("xp", [128, KH * M], F16, kind="ExternalInput")
    wcp = nc.dram_tensor("wcp", [128, KH * 384], F16, kind="ExternalInput")
    wqk2 = nc.dram_tensor("wqk2", [128, H_LOC * 4 * 128], F16,
                          kind="ExternalInput")
    kvp = nc.dram_tensor("kvp", [128, 4 * S_KV], F16, kind="ExternalInput")
    kpe2d = nc.dram_tensor("kpe2d", [128, S_KV], F16, kind="ExternalInput")
    cntp = nc.dram_tensor("cntp", [128, 2 * NPR * M], F16,
                          kind="ExternalInput")
    vwp = nc.dram_tensor("vwp", [128, H_LOC * NSC * 128], F16,
                         kind="ExternalInput")
    wopp = nc.dram_tensor("wopp", [128, 16 * OUT_C], F16,
                          kind="ExternalInput")
    outT = nc.dram_tensor("outT", [OUT_C, M], F32, kind="ExternalOutput")

    rg = [list(range(N_CORES))]

    with tile.TileContext(nc) as tc, \
            nc.allow_low_precision(reason="f16 matmul pipeline, f32 accum"):
        with tc.tile_pool(name="dram", bufs=1, space="DRAM") as dram:
            o2l = [dram.tile([H_LOC * 128, MH], F16, name=f"o2l{mh}")
                   for mh in range(2)]
            o2a = [dram.tile([H_LOC * 128 * N_CORES, MH], F16,
                             name=f"o2a{mh}", addr_space="Shared")
                   for mh in range(2)]

            per_cm = tc.tile_pool(name="per", bufs=1)
            per = per_cm.__enter__()
            kabs = [per.tile([128, S_KV], F16, name=f"kabs{h}")
                    for h in range(H_LOC)]
            kpe_sb = per.tile([128, S_KV], F16, name="kpe")
            qn = [per.tile([128, M], F16, name=f"qn{h}")
                  for h in range(H_LOC)]
            qpe2 = per.tile([128, M], F16, name="qpe2")
            vw_sb = per.tile([128, H_LOC * NSC * 128], F16, name="vw")
            wop_sb = per.tile([128, 16 * OUT_C], F16, name="wop")
            ones_col = per.tile([128, 1], F16, name="ones_col")
            ones_row = per.tile([1, 128], F32, name="ones_row")
            nc.vector.memset(ones_col[:], 1.0)
            nc.vector.memset(ones_row[:], 1.0)

            s1_cm = tc.tile_pool(name="s1", bufs=1)
            s1 = s1_cm.__enter__()
            warm = s1.tile([128, 64], F32, name="warm")
            nc.vector.memset(warm[:], 0.0)
            kv_sb = s1.tile([128, 4 * S_KV], F16, name="kv")
            wqk_sb = s1.tile([128, H_LOC * 4 * 128], F16, name="wqk")

            # ---------------- stage 1+2 fused: q = x @ W_comb ------------
            # (first so its DMA stream overlaps the collectives' init)
            with (
                tc.tile_pool(name="s2x", bufs=3) as s2x,
                tc.tile_pool(name="s2w", bufs=3) as s2w,
                tc.tile_pool(name="psq", bufs=1, space="PSUM") as psq,
                tc.tile_pool(name="psw", bufs=1, space="PSUM") as psw,
            ):
                wps = psw.tile([1, 64], F32, name="wps", tag="wps")
                for i in range(N_WARM):
                    nc.tensor.matmul(wps[:], warm[:, 0:1], warm[:],
                                     start=(i == 0), stop=(i == N_WARM - 1),
                                     skip_group_check=True)
                pq = [psq.tile([128, M], F32, name=f"q{j}", tag=f"q{j}")
                      for j in range(3)]
                NG = KH // 8   # 7 supergroups of 8 hid-chunks
                sg = []
                for g in range(NG):
                    wt = s2w.tile([128, 8 * 384], F16, name="wt", tag="wt")
                    nc.sync.dma_start(
                        wt[:], wcp[:, g * 8 * 384:(g + 1) * 8 * 384])
                    xa = s2x.tile([128, 2048], F16, name="xa", tag="xa")
                    nc.sync.dma_start(
                        xa[:], xp[:, (8 * g) * M:(8 * g + 4) * M])
                    xb = s2x.tile([128, 2048], F16, name="xb", tag="xb")
                    nc.sync.dma_start(
                        xb[:], xp[:, (8 * g + 4) * M:(8 * g + 8) * M])
                    if 1 <= g <= 4:
                        # spread k_abs input loads between supergroups
                        pcs = [(0, 0), (1, 0), (2, 0), (3, 0),
                               (0, 1), (1, 1), (2, 1), (3, 1)]
                        for c, half in pcs[(g - 1) * 2:(g - 1) * 2 + 2]:
                            nc.sync.dma_start(
                                kv_sb[:, c * S_KV + half * 2048:
                                      c * S_KV + (half + 1) * 2048],
                                kvp[:, c * S_KV + half * 2048:
                                    c * S_KV + (half + 1) * 2048])
                    if g == 5:
                        nc.sync.dma_start(wqk_sb[:], wqk2[:])
                        nc.sync.dma_start(kpe_sb[:, 0:2048],
                                          kpe2d[:, 0:2048])
                        nc.sync.dma_start(kpe_sb[:, 2048:4096],
                                          kpe2d[:, 2048:4096])
                    for kk in range(8):
                        xt = xa if kk < 4 else xb
                        rhs = xt[:, (kk % 4) * M:(kk % 4 + 1) * M]
                        for j in range(3):
                            nc.tensor.matmul(
                                pq[j][:],
                                wt[:, kk * 384 + j * 128:
                                   kk * 384 + (j + 1) * 128],
                                rhs,
                                start=(g == 0 and kk == 0),
                                stop=(g == NG - 1 and kk == 7),
                                skip_group_check=True)
                nc.vector.tensor_copy(qn[0][:], pq[0][:])
                nc.scalar.copy(qn[1][:], pq[1][:])
                nc.vector.tensor_copy(qpe2[:], pq[2][:])

            # ---------------- k_abs --------------------------------------
            with tc.tile_pool(name="psk", bufs=2, space="PSUM") as psk:
                nc.sync.dma_start(vw_sb[:, 0:S_KV], vwp[:, 0:S_KV])
                nc.sync.dma_start(vw_sb[:, S_KV:2 * S_KV],
                                  vwp[:, S_KV:2 * S_KV])
                for pc in range(8):
                    for h in range(H_LOC):
                        kps = psk.tile([128, 512], F32, name="kps",
                                       tag="kps")
                        for c in range(4):
                            nc.tensor.matmul(
                                kps[:],
                                wqk_sb[:, (h * 4 + c) * 128:
                                       (h * 4 + c + 1) * 128],
                                kv_sb[:, c * S_KV + pc * 512:
                                      c * S_KV + (pc + 1) * 512],
                                start=(c == 0), stop=(c == 3))
                        if h == 0:
                            nc.vector.tensor_copy(
                                kabs[h][:, pc * 512:(pc + 1) * 512], kps[:])
                        else:
                            nc.scalar.copy(
                                kabs[h][:, pc * 512:(pc + 1) * 512], kps[:])
            s1_cm.__exit__(None, None, None)

            # ---------------- attention over m-halves --------------------
            for mh in range(2):
                m0 = mh * MH
                with (
                    tc.tile_pool(name=f"cnt{mh}", bufs=8) as cnts,
                    tc.tile_pool(name=f"exs{mh}", bufs=4) as exs,
                    tc.tile_pool(name=f"pts{mh}", bufs=4) as pts,
                    tc.tile_pool(name=f"psS{mh}", bufs=LAG,
                                 space="PSUM") as psS,
                    tc.tile_pool(name=f"psZ{mh}", bufs=1,
                                 space="PSUM") as psZ,
                ):
                    o2p = psZ.tile([128, 2 * MH], F32, name="o2p")
                    z_ps = [psZ.tile([1, M], F32, name=f"zp{h}")
                            for h in range(H_LOC)]
                    cnt_t = {}
                    for pr2 in range(NPR // 2):
                        t = cnts.tile([128, 2 * M], F16, name="cc",
                                      tag="cc")
                        nc.sync.dma_start(
                            t[:], cntp[:, (mh * NPR + 2 * pr2) * M:
                                       (mh * NPR + 2 * pr2 + 2) * M])
                        cnt_t[2 * pr2] = t[:, 0:M]
                        cnt_t[2 * pr2 + 1] = t[:, M:2 * M]
                    if mh == 0:
                        for w8 in range(8):
                            nc.sync.dma_start(
                                wop_sb[:, w8 * 2 * OUT_C:
                                       (w8 + 1) * 2 * OUT_C],
                                wopp[:, w8 * 2 * OUT_C:
                                     (w8 + 1) * 2 * OUT_C])

                    pend = {}

                    def emit_scores(pr):
                        ps = psS.tile([128, 4 * MH], F32, name="ss",
                                      tag="ss")
                        # one start per bank (h0 -> bank A, h1 -> bank B)
                        for q in range(2):
                            sc = 2 * pr + q
                            for h in range(H_LOC):
                                nc.tensor.matmul(
                                    ps[:, (h * 2 + q) * MH:
                                       (h * 2 + q + 1) * MH],
                                    kabs[h][:, sc * 128:(sc + 1) * 128],
                                    qn[h][:, m0:m0 + MH],
                                    start=(q == 0), stop=False,
                                    skip_group_check=True)
                        for q in range(2):
                            sc = 2 * pr + q
                            for h in range(H_LOC):
                                b = h * 64
                                nc.tensor.matmul(
                                    ps[:, (h * 2 + q) * MH:
                                       (h * 2 + q + 1) * MH],
                                    kpe_sb[b:b + 64,
                                           sc * 128:(sc + 1) * 128],
                                    qpe2[b:b + 64, m0:m0 + MH],
                                    start=False, stop=True,
                                    skip_group_check=True,
                                    tile_position=(b, 0))
                        pend[pr] = ps

                    def consume(pr):
                        ps = pend.pop(pr)
                        ex = exs.tile([128, 4 * MH], F16, name="ex",
                                      tag="ex")
                        nc.scalar.activation(
                            ex[:], ps[:],
                            mybir.ActivationFunctionType.Exp,
                            scale=SM_SCALE)
                        for h in range(H_LOC):
                            pt = pts.tile([128, 2 * MH], F16, name="pt",
                                          tag="pt")
                            nc.vector.tensor_mul(
                                pt[:], ex[:, h * 2 * MH:(h + 1) * 2 * MH],
                                cnt_t[pr])
                            nc.tensor.matmul(
                                z_ps[h][:], ones_col[:], pt[:],
                                start=(pr == 0), stop=(pr == NPR - 1),
                                skip_group_check=True)
                            for q in range(2):
                                sc = 2 * pr + q
                                nc.tensor.matmul(
                                    o2p[:, h * MH:(h + 1) * MH],
                                    vw_sb[:, (h * NSC + sc) * 128:
                                          (h * NSC + sc + 1) * 128],
                                    pt[:, q * MH:(q + 1) * MH],
                                    start=(pr == 0 and q == 0 and h == 0),
                                    stop=(pr == NPR - 1 and q == 1),
                                    skip_group_check=True)

                    for pr in range(NPR):
                        emit_scores(pr)
                        if pr >= LAG:
                            consume(pr - LAG)
                    for pr in range(NPR - LAG, NPR):
                        consume(pr)

                    # Z -> 1/Z broadcast -> normalize -> AllGather
                    zs_l = []
                    for h in range(H_LOC):
                        zsb = exs.tile([1, M], F32, name="zsb", tag="zsb")
                        nc.scalar.copy(zsb[:], z_ps[h][:])
                        zs = exs.tile([1, MH], F32, name="zs", tag="zs")
                        nc.vector.tensor_add(zs[:], zsb[0:1, 0:MH],
                                             zsb[0:1, MH:2 * MH])
                        zs_l.append(zs)
                    zbt = psS.tile([128, 4 * MH], F32, name="ss", tag="ss")
                    nc.tensor.matmul(zbt[:, 0:MH], ones_row[:],
                                     zs_l[0][:], start=True, stop=True,
                                     skip_group_check=True)
                    nc.tensor.matmul(zbt[:, MH:2 * MH], ones_row[:],
                                     zs_l[1][:], start=False, stop=True,
                                     skip_group_check=True)
                    zbsb = exs.tile([128, 2 * MH], F32, name="zbsb",
                                    tag="zbsb")
                    nc.scalar.copy(zbsb[:], zbt[:, 0:2 * MH])
                    rzb = exs.tile([128, 2 * MH], F32, name="rzb",
                                   tag="rzb")
                    nc.vector.reciprocal_approx_fast(rzb[:], zbsb[:])
                    for h in range(H_LOC):
                        o2s = pts.tile([128, MH], F16, name=f"o2s{h}")
                        nc.vector.tensor_mul(
                            o2s[:], o2p[:, h * MH:(h + 1) * MH],
                            rzb[:, h * MH:(h + 1) * MH])
                        nc.sync.dma_start(
                            o2l[mh][h * 128:(h + 1) * 128, :], o2s[:])
                    nc.gpsimd.collective_compute(
                        "AllGather", mybir.AluOpType.bypass,
                        replica_groups=rg,
                        ins=[o2l[mh].opt()], outs=[o2a[mh].opt()])
                    for f in range(6 if mh == 0 else 0):
                        nc.tensor.matmul(
                            zbt[:, 2 * MH:4 * MH],
                            vw_sb[:, 0:128], vw_sb[:, 0:M],
                            start=(f == 0), stop=True,
                            skip_group_check=True)

            # ---------------- O-projection (per half, p-outer) -----------
            for mh in range(2):
                with (
                    tc.tile_pool(name=f"s6{mh}", bufs=1) as s6,
                    tc.tile_pool(name=f"ps6{mh}", bufs=2,
                                 space="PSUM") as ps6,
                    tc.tile_pool(name=f"s6o{mh}", bufs=4) as s6o,
                ):
                    o2t = []
                    for n in range(16):
                        ok = s6.tile([128, MH], F16, name=f"o2t{n}")
                        nc.sync.dma_start(
                            ok[:], o2a[mh][n * 128:(n + 1) * 128, :])
                        o2t.append(ok)
                    fill = ps6.tile([128, 512], F32, name="pb", tag="pb")
                    for f in range(40 if mh == 0 else 14):
                        nc.tensor.matmul(
                            fill[:, 0:M], vw_sb[:, 0:128], vw_sb[:, 0:M],
                            start=(f == 0), stop=True,
                            skip_group_check=True)
                    for p in range(OUT_C // 128):
                        pb = ps6.tile([128, 512], F32, name="pb", tag="pb")
                        for n, ok in enumerate(o2t):
                            nc.tensor.matmul(
                                pb[:, 0:MH],
                                wop_sb[:, n * OUT_C + p * 128:
                                       n * OUT_C + (p + 1) * 128],
                                ok[:], start=(n == 0), stop=(n == 15),
                                skip_group_check=True)
                        ob = s6o.tile([128, MH], F32, name="outb",
                                      tag="outb")
                        if p % 2 == 0:
                            nc.vector.tensor_copy(ob[:], pb[:, 0:MH])
                        else:
                            nc.scalar.copy(ob[:], pb[:, 0:MH])
                        nc.sync.dma_start(
                            outT[p * 128:(p + 1) * 128,
                                 mh * MH:(mh + 1) * MH], ob[:])

            per_cm.__exit__(None, None, None)

    nc.compile()
    return nc


def prep_inputs(x, W_cqkv, W_uq, W_qk, kv_cache, W_o1, W_oproj, indices):
    x = np.asarray(x, np.float32)
    W_cqkv = np.asarray(W_cqkv, np.float32)
    W_uq = np.asarray(W_uq, np.float32)
    W_qk = np.asarray(W_qk, np.float32)
    kv_cache = np.asarray(kv_cache, np.float32)
    W_o1 = np.asarray(W_o1, np.float32)
    W_oproj = np.asarray(W_oproj, np.float32)
    indices = np.asarray(indices)

    # host-side algebraic folds (f32)
    w_comb = W_cqkv[:, D_KV_C:D_KV_C + D_Q_C] @ W_uq        # [7168, 3072]
    vw = np.einsum("sc,hcv->hsv", kv_cache[:, :D_KV_C], W_o1)  # [16,4096,128]

    def pack(a, nchunk):
        # [nchunk*128, F] -> [128, nchunk*F]
        f = a.shape[1]
        return np.ascontiguousarray(
            a.reshape(nchunk, 128, f).transpose(1, 0, 2).reshape(
                128, nchunk * f))

    kvT = kv_cache.T                                         # [576, 4096]
    kvp = pack(kvT[:D_KV_C], 4).astype(np.float16)
    kpe2 = np.concatenate([kvT[D_KV_C:], kvT[D_KV_C:]], 0).astype(np.float16)

    cm = np.zeros((M, S_KV), np.float32)
    np.add.at(cm, (np.arange(M)[:, None], indices), 1.0)
    # cnt_pack[p, mh, pr, q, m256] = cm[mh*256+m, 128*(2pr+q)+p]
    cmT = np.ascontiguousarray(cm.T).reshape(NSC, 128, 2, MH)
    cntp = np.empty((128, 2, NPR, 2, MH), np.float32)
    for pr in range(NPR):
        for q in range(2):
            sc = 2 * pr + q
            for mhh in range(2):
                cntp[:, mhh, pr, q, :] = cmT[sc, :, mhh, :]
    cntp = np.ascontiguousarray(
        cntp.reshape(128, 2 * NPR * M)).astype(np.float16)

    xpk = pack(np.ascontiguousarray(x.T), KH).astype(np.float16)

    in_maps = []
    for i in range(N_CORES):
        h0 = i * H_LOC
        c0 = i * OUT_C
        cols = np.concatenate([
            w_comb[:, (h0 + 0) * 192:(h0 + 0) * 192 + 128],
            w_comb[:, (h0 + 1) * 192:(h0 + 1) * 192 + 128],
            w_comb[:, (h0 + 0) * 192 + 128:(h0 + 1) * 192],
            w_comb[:, (h0 + 1) * 192 + 128:(h0 + 2) * 192],
        ], axis=1)                                           # [7168, 384]
        wcpk = pack(cols, KH).astype(np.float16)

        wqk_l = np.stack([
            pack(np.ascontiguousarray(W_qk[h].T), 4)
            for h in range(h0, h0 + H_LOC)], axis=1)
        wqk_l = np.ascontiguousarray(
            wqk_l.reshape(128, H_LOC * 4 * 128)).astype(np.float16)

        vw_l = np.stack([pack(vw[h], NSC)
                         for h in range(h0, h0 + H_LOC)], axis=1)
        vw_l = np.ascontiguousarray(
            vw_l.reshape(128, H_LOC * NSC * 128)).astype(np.float16)

        # gathered chunk n = rank n//2, local head n%2 = global head n
        order = [2 * k + h for k in range(N_CORES) for h in range(H_LOC)]
        wop_r = W_oproj.reshape(16, 128, HID)[order][:, :, c0:c0 + OUT_C]
        wop_l = pack(wop_r.reshape(16 * 128, OUT_C), 16).astype(np.float16)

        in_maps.append({
            "xp": xpk,
            "wcp": wcpk,
            "wqk2": wqk_l,
            "kvp": kvp,
            "kpe2d": kpe2,
            "cntp": cntp,
            "vwp": vw_l,
            "wopp": wop_l,
        })
    return in_maps


_prog_cache = {}


def kernel(x, W_cqkv, W_uq, W_qk, kv_cache, W_o1, W_oproj, indices):
    if "nc" not in _prog_cache:
        _prog_cache["nc"] = build_program()
    nc = _prog_cache["nc"]
    in_maps = prep_inputs(x, W_cqkv, W_uq, W_qk, kv_cache, W_o1, W_oproj,
                          indices)
    trace = bool(int(os.environ.get("KERNEL_TRACE", "0")))
    res = run_bass_kernel_spmd(nc, in_maps, list(range(N_CORES)),
                               trace=trace)
    _prog_cache["last_result"] = res
    out = np.empty((M, HID), np.float32)
    for i in range(N_CORES):
        out[:, i * OUT_C:(i + 1) * OUT_C] = res.results[i]["outT"].T
    return out


# revision 23
# speedup vs baseline: 1.0976x; 1.0441x over previous
"""DeepSeek MLA prefill (absorbed) on 8 Trainium2 NeuronCores — v4.

Sharding: tensor-parallel over heads (2 of 16 per core). Host-side
algebraic folds remove both the QKV-compression collective and most of
the attention FLOPs:
  - W_comb = W_cqkv[:, qc] @ W_uq   -> q computed straight from x.
  - k_abs = kv_c @ W_qk[h].T on device (192-dim score contraction).
  - VW[h] = V @ W_o1[h] on host     -> value+O-bmm in one matmul.
Top-k selection = dense count-weighted softmax. All f16, f32 PSUM.

Phases: q-GEMM streams x/W_comb first (overlapping the collectives'
init window), then k_abs, then attention twice over 256-token halves
so half A's o2 AllGather rides under half B's compute and the
O-projection pipelines under half B's AllGather. Inside a half,
sc-chunks run in pairs three-deep pipelined: scores (PE) -> exp (ACT,
one op per pair) -> xcnt (DVE) -> Z + value (PE). Z for both heads
packs into one PSUM bank (single start per bank; has_written handles
the second group).
"""

import os
import sys

sys.path.insert(0, "/opt/trn_rl_repo")

import numpy as np

import concourse.bass as bass
import concourse.tile as tile
from concourse import bacc, mybir
from concourse.bass_utils import run_bass_kernel_spmd

F32 = mybir.dt.float32
F16 = mybir.dt.float16

N_CORES = 8
M = 512
MH = M // 2
HID = 7168
D_KV_C, D_Q_C, D_R, D_Q = 512, 1536, 64, 128
H_LOC = 2
D_ATT = D_KV_C + D_R
S_KV = 4096
OUT_C = HID // N_CORES          # 896
KH = HID // 128                 # 56
NSC = S_KV // 128               # 32
NPR = NSC // 2                  # 16 sc-pairs
SM_SCALE = 1.0 / float(np.sqrt(np.float32(D_ATT)))
N_WARM = 48
LAG = 2


def build_program():
    nc = bacc.Bacc("TRN2", target_bir_lowering=False, debug=False,
                   num_devices=N_CORES)

    xp = nc.dram_tensor("xp", [128, KH * M], F16, kind="ExternalInput")
    wcp = nc.dram_tensor("wcp", [128, KH * 384], F16, kind="ExternalInput")
    wqk2 = nc.dram_tensor("wqk2", [128, H_LOC * 4 * 128], F16,
                          kind="ExternalInput")
    kvp = nc.dram_tensor("kvp", [128, 4 * S_KV], F16, kind="ExternalInput")
    kpe2d = nc.dram_tensor("kpe2d", [128, S_KV], F16, kind="ExternalInput")
    cntp = nc.dram_tensor("cntp", [128, 2 * NPR * M], F16,
                          kind="ExternalInput")
    vwp = nc.dram_tensor("vwp", [128, H_LOC * NSC * 128], F16,
                         kind="ExternalInput")
    wopp = nc.dram_tensor("wopp", [128, 16 * OUT_C], F16,
                          kind="ExternalInput")
    outT = nc.dram_tensor("outT", [OUT_C, M], F32, kind="ExternalOutput")

    rg = [list(range(N_CORES))]

    with tile.TileContext(nc) as tc, \
            nc.allow_low_precision(reason="f16 matmul pipeline, f32 accum"):
        with tc.tile_pool(name="dram", bufs=1, space="DRAM") as dram:
            o2l = [dram.tile([H_LOC * 128, MH], F16, name=f"o2l{mh}")
                   for mh in range(2)]
            o2a = [dram.tile([H_LOC * 128 * N_CORES, MH], F16,
                             name=f"o2a{mh}", addr_space="Shared")
                   for mh in range(2)]

            per_cm = tc.tile_pool(name="per", bufs=1)
            per = per_cm.__enter__()
            kabs = [per.tile([128, S_KV], F16, name=f"kabs{h}")
                    for h in range(H_LOC)]
            kpe_sb = per.tile([128, S_KV], F16, name="kpe")
            qn = [per.tile([128, M], F16, name=f"qn{h}")
                  for h in range(H_LOC)]
            qpe2 = per.tile([128, M], F16, name="qpe2")
            vw_sb = per.tile([128, H_LOC * NSC * 128], F16, name="vw")
            wop_sb = per.tile([128, 16 * OUT_C], F16, name="wop")
            ones_col = per.tile([128, 1], F16, name="ones_col")
            ones_row = per.tile([1, 128], F32, name="ones_row")
            nc.vector.memset(ones_col[:], 1.0)
            nc.vector.memset(ones_row[:], 1.0)

            s1_cm = tc.tile_pool(name="s1", bufs=1)
            s1 = s1_cm.__enter__()
            warm = s1.tile([128, 64], F32, name="warm")
            nc.vector.memset(warm[:], 0.0)
            kv_sb = s1.tile([128, 4 * S_KV], F16, name="kv")
            wqk_sb = s1.tile([128, H_LOC * 4 * 128], F16, name="wqk")

            # ---------------- stage 1+2 fused: q = x @ W_comb ------------
            # (first so its DMA stream overlaps the collectives' init)
            with (
                tc.tile_pool(name="s2x", bufs=3) as s2x,
                tc.tile_pool(name="s2w", bufs=3) as s2w,
                tc.tile_pool(name="psq", bufs=1, space="PSUM") as psq,
                tc.tile_pool(name="psw", bufs=1, space="PSUM") as psw,
            ):
                wps = psw.tile([1, 64], F32, name="wps", tag="wps")
                for i in range(N_WARM):
                    nc.tensor.matmul(wps[:], warm[:, 0:1], warm[:],
                                     start=(i == 0), stop=(i == N_WARM - 1),
                                     skip_group_check=True)
                pq = [psq.tile([128, M], F32, name=f"q{j}", tag=f"q{j}")
                      for j in range(3)]
                NG = KH // 8   # 7 supergroups of 8 hid-chunks
                sg = []
                for g in range(NG):
                    wt = s2w.tile([128, 8 * 384], F16, name="wt", tag="wt")
                    nc.sync.dma_start(
                        wt[:], wcp[:, g * 8 * 384:(g + 1) * 8 * 384])
                    xa = s2x.tile([128, 2048], F16, name="xa", tag="xa")
                    nc.sync.dma_start(
                        xa[:], xp[:, (8 * g) * M:(8 * g + 4) * M])
                    xb = s2x.tile([128, 2048], F16, name="xb", tag="xb")
                    nc.sync.dma_start(
                        xb[:], xp[:, (8 * g + 4) * M:(8 * g + 8) * M])
                    if 1 <= g <= 4:
                        # spread k_abs input loads between supergroups
                        pcs = [(0, 0), (1, 0), (2, 0), (3, 0),
                               (0, 1), (1, 1), (2, 1), (3, 1)]
                        for c, half in pcs[(g - 1) * 2:(g - 1) * 2 + 2]:
                            nc.sync.dma_start(
                                kv_sb[:, c * S_KV + half * 2048:
                                      c * S_KV + (half + 1) * 2048],
                                kvp[:, c * S_KV + half * 2048:
                                    c * S_KV + (half + 1) * 2048])
                    if g == 5:
                        nc.sync.dma_start(wqk_sb[:], wqk2[:])
                        nc.sync.dma_start(kpe_sb[:, 0:2048],
                                          kpe2d[:, 0:2048])
                        nc.sync.dma_start(kpe_sb[:, 2048:4096],
                                          kpe2d[:, 2048:4096])
                    for kk in range(8):
                        xt = xa if kk < 4 else xb
                        rhs = xt[:, (kk % 4) * M:(kk % 4 + 1) * M]
                        for j in range(3):
                            nc.tensor.matmul(
                                pq[j][:],
                                wt[:, kk * 384 + j * 128:
                                   kk * 384 + (j + 1) * 128],
                                rhs,
                                start=(g == 0 and kk == 0),
                                stop=(g == NG - 1 and kk == 7),
                                skip_group_check=True)
                nc.vector.tensor_copy(qn[0][:], pq[0][:])
                nc.scalar.copy(qn[1][:], pq[1][:])
                nc.vector.tensor_copy(qpe2[:], pq[2][:])

            # ---------------- k_abs --------------------------------------
            with tc.tile_pool(name="psk", bufs=2, space="PSUM") as psk:
                nc.sync.dma_start(vw_sb[:, 0:S_KV], vwp[:, 0:S_KV])
                nc.sync.dma_start(vw_sb[:, S_KV:2 * S_KV],
                                  vwp[:, S_KV:2 * S_KV])
                for w8 in range(8):
                    nc.sync.dma_start(
                        wop_sb[:, w8 * 2 * OUT_C:(w8 + 1) * 2 * OUT_C],
                        wopp[:, w8 * 2 * OUT_C:(w8 + 1) * 2 * OUT_C])
                for pc in range(8):
                    for h in range(H_LOC):
                        kps = psk.tile([128, 512], F32, name="kps",
                                       tag="kps")
                        for c in range(4):
                            nc.tensor.matmul(
                                kps[:],
                                wqk_sb[:, (h * 4 + c) * 128:
                                       (h * 4 + c + 1) * 128],
                                kv_sb[:, c * S_KV + pc * 512:
                                      c * S_KV + (pc + 1) * 512],
                                start=(c == 0), stop=(c == 3))
                        if h == 0:
                            nc.vector.tensor_copy(
                                kabs[h][:, pc * 512:(pc + 1) * 512], kps[:])
                        else:
                            nc.scalar.copy(
                                kabs[h][:, pc * 512:(pc + 1) * 512], kps[:])
            s1_cm.__exit__(None, None, None)

            # ---------------- attention over m-halves --------------------
            for mh in range(2):
                m0 = mh * MH
                with (
                    tc.tile_pool(name=f"cnt{mh}", bufs=4) as cnts,
                    tc.tile_pool(name=f"exs{mh}", bufs=4) as exs,
                    tc.tile_pool(name=f"pts{mh}", bufs=4) as pts,
                    tc.tile_pool(name=f"psS{mh}", bufs=LAG,
                                 space="PSUM") as psS,
                    tc.tile_pool(name=f"psZ{mh}", bufs=1,
                                 space="PSUM") as psZ,
                ):
                    o2p = psZ.tile([128, 2 * MH], F32, name="o2p")
                    z_ps = [psZ.tile([1, M], F32, name=f"zp{h}")
                            for h in range(H_LOC)]
                    cnt_t = {}

                    def cnt_load(pr2):
                        t = cnts.tile([128, 2 * M], F16, name="cc",
                                      tag="cc")
                        nc.sync.dma_start(
                            t[:], cntp[:, (mh * NPR + 2 * pr2) * M:
                                       (mh * NPR + 2 * pr2 + 2) * M])
                        cnt_t[2 * pr2] = t[:, 0:M]
                        cnt_t[2 * pr2 + 1] = t[:, M:2 * M]

                    cnt_load(0)
                    cnt_load(1)

                    pend = {}

                    def emit_scores(pr):
                        ps = psS.tile([128, 4 * MH], F32, name="ss",
                                      tag="ss")
                        # one start per bank (h0 -> bank A, h1 -> bank B)
                        for q in range(2):
                            sc = 2 * pr + q
                            for h in range(H_LOC):
                                nc.tensor.matmul(
                                    ps[:, (h * 2 + q) * MH:
                                       (h * 2 + q + 1) * MH],
                                    kabs[h][:, sc * 128:(sc + 1) * 128],
                                    qn[h][:, m0:m0 + MH],
                                    start=(q == 0), stop=False,
                                    skip_group_check=True)
                        for q in range(2):
                            sc = 2 * pr + q
                            for h in range(H_LOC):
                                b = h * 64
                                nc.tensor.matmul(
                                    ps[:, (h * 2 + q) * MH:
                                       (h * 2 + q + 1) * MH],
                                    kpe_sb[b:b + 64,
                                           sc * 128:(sc + 1) * 128],
                                    qpe2[b:b + 64, m0:m0 + MH],
                                    start=False, stop=True,
                                    skip_group_check=True,
                                    tile_position=(b, 0))
                        pend[pr] = ps

                    def consume(pr):
                        ps = pend.pop(pr)
                        ex = exs.tile([128, 4 * MH], F16, name="ex",
                                      tag="ex")
                        nc.scalar.activation(
                            ex[:], ps[:],
                            mybir.ActivationFunctionType.Exp,
                            scale=SM_SCALE)
                        for h in range(H_LOC):
                            pt = pts.tile([128, 2 * MH], F16, name="pt",
                                          tag="pt")
                            nc.vector.tensor_mul(
                                pt[:], ex[:, h * 2 * MH:(h + 1) * 2 * MH],
                                cnt_t[pr])
                            nc.tensor.matmul(
                                z_ps[h][:], ones_col[:], pt[:],
                                start=(pr == 0), stop=(pr == NPR - 1),
                                skip_group_check=True)
                            for q in range(2):
                                sc = 2 * pr + q
                                nc.tensor.matmul(
                                    o2p[:, h * MH:(h + 1) * MH],
                                    vw_sb[:, (h * NSC + sc) * 128:
                                          (h * NSC + sc + 1) * 128],
                                    pt[:, q * MH:(q + 1) * MH],
                                    start=(pr == 0 and q == 0 and h == 0),
                                    stop=(pr == NPR - 1 and q == 1),
                                    skip_group_check=True)

                    for pr in range(NPR):
                        if pr % 2 == 0 and pr // 2 + 2 < NPR // 2:
                            cnt_load(pr // 2 + 2)
                        emit_scores(pr)
                        if pr >= LAG:
                            consume(pr - LAG)
                    for pr in range(NPR - LAG, NPR):
                        consume(pr)

                    # Z -> 1/Z broadcast -> normalize -> AllGather
                    zs_l = []
                    for h in range(H_LOC):
                        zsb = exs.tile([1, M], F32, name="zsb", tag="zsb")
                        nc.scalar.copy(zsb[:], z_ps[h][:])
                        zs = exs.tile([1, MH], F32, name="zs", tag="zs")
                        nc.vector.tensor_add(zs[:], zsb[0:1, 0:MH],
                                             zsb[0:1, MH:2 * MH])
                        zs_l.append(zs)
                    zbt = psS.tile([128, 4 * MH], F32, name="ss", tag="ss")
                    nc.tensor.matmul(zbt[:, 0:MH], ones_row[:],
                                     zs_l[0][:], start=True, stop=True,
                                     skip_group_check=True)
                    nc.tensor.matmul(zbt[:, MH:2 * MH], ones_row[:],
                                     zs_l[1][:], start=False, stop=True,
                                     skip_group_check=True)
                    zbsb = exs.tile([128, 2 * MH], F32, name="zbsb",
                                    tag="zbsb")
                    nc.scalar.copy(zbsb[:], zbt[:, 0:2 * MH])
                    rzb = exs.tile([128, 2 * MH], F32, name="rzb",
                                   tag="rzb")
                    nc.vector.reciprocal_approx_fast(rzb[:], zbsb[:])
                    for h in range(H_LOC):
                        o2s = pts.tile([128, MH], F16, name=f"o2s{h}")
                        nc.vector.tensor_mul(
                            o2s[:], o2p[:, h * MH:(h + 1) * MH],
                            rzb[:, h * MH:(h + 1) * MH])
                        nc.sync.dma_start(
                            o2l[mh][h * 128:(h + 1) * 128, :], o2s[:])
                    nc.gpsimd.collective_compute(
                        "AllGather", mybir.AluOpType.bypass,
                        replica_groups=rg,
                        ins=[o2l[mh].opt()], outs=[o2a[mh].opt()])
                    for f in range(6 if mh == 0 else 0):
                        nc.tensor.matmul(
                            zbt[:, 2 * MH:4 * MH],
                            vw_sb[:, 0:128], vw_sb[:, 0:M],
                            start=(f == 0), stop=True,
                            skip_group_check=True)

            # ---------------- O-projection (per half, p-outer) -----------
            for mh in range(2):
                with (
                    tc.tile_pool(name=f"s6{mh}", bufs=1) as s6,
                    tc.tile_pool(name=f"ps6{mh}", bufs=2,
                                 space="PSUM") as ps6,
                    tc.tile_pool(name=f"s6o{mh}", bufs=4) as s6o,
                ):
                    o2t = []
                    for n in range(16):
                        ok = s6.tile([128, MH], F16, name=f"o2t{n}")
                        nc.sync.dma_start(
                            ok[:], o2a[mh][n * 128:(n + 1) * 128, :])
                        o2t.append(ok)
                    fill = ps6.tile([128, 512], F32, name="pb", tag="pb")
                    for f in range(40 if mh == 0 else 14):
                        nc.tensor.matmul(
                            fill[:, 0:M], vw_sb[:, 0:128], vw_sb[:, 0:M],
                            start=(f == 0), stop=True,
                            skip_group_check=True)
                    for p in range(OUT_C // 128):
                        pb = ps6.tile([128, 512], F32, name="pb", tag="pb")
                        for n, ok in enumerate(o2t):
                            nc.tensor.matmul(
                                pb[:, 0:MH],
                                wop_sb[:, n * OUT_C + p * 128:
                                       n * OUT_C + (p + 1) * 128],
                                ok[:], start=(n == 0), stop=(n == 15),
                                skip_group_check=True)
                        ob = s6o.tile([128, MH], F32, name="outb",
                                      tag="outb")
                        if p % 2 == 0:
                            nc.vector.tensor_copy(ob[:], pb[:, 0:MH])
                        else:
                            nc.scalar.copy(ob[:], pb[:, 0:MH])
                        nc.sync.dma_start(
                            outT[p * 128:(p + 1) * 128,
                                 mh * MH:(mh + 1) * MH], ob[:])

            per_cm.__exit__(None, None, None)

    nc.compile()
    return nc


def prep_inputs(x, W_cqkv, W_uq, W_qk, kv_cache, W_o1, W_oproj, indices):
    x = np.asarray(x, np.float32)
    W_cqkv = np.asarray(W_cqkv, np.float32)
    W_uq = np.asarray(W_uq, np.float32)
    W_qk = np.asarray(W_qk, np.float32)
    kv_cache = np.asarray(kv_cache, np.float32)
    W_o1 = np.asarray(W_o1, np.float32)
    W_oproj = np.asarray(W_oproj, np.float32)
    indices = np.asarray(indices)

    # host-side algebraic folds (f32)
    w_comb = W_cqkv[:, D_KV_C:D_KV_C + D_Q_C] @ W_uq        # [7168, 3072]
    vw = np.einsum("sc,hcv->hsv", kv_cache[:, :D_KV_C], W_o1)  # [16,4096,128]

    def pack(a, nchunk):
        # [nchunk*128, F] -> [128, nchunk*F]
        f = a.shape[1]
        return np.ascontiguousarray(
            a.reshape(nchunk, 128, f).transpose(1, 0, 2).reshape(
                128, nchunk * f))

    kvT = kv_cache.T                                         # [576, 4096]
    kvp = pack(kvT[:D_KV_C], 4).astype(np.float16)
    kpe2 = np.concatenate([kvT[D_KV_C:], kvT[D_KV_C:]], 0).astype(np.float16)

    cm = np.zeros((M, S_KV), np.float32)
    np.add.at(cm, (np.arange(M)[:, None], indices), 1.0)
    # cnt_pack[p, mh, pr, q, m256] = cm[mh*256+m, 128*(2pr+q)+p]
    cmT = np.ascontiguousarray(cm.T).reshape(NSC, 128, 2, MH)
    cntp = np.empty((128, 2, NPR, 2, MH), np.float32)
    for pr in range(NPR):
        for q in range(2):
            sc = 2 * pr + q
            for mhh in range(2):
                cntp[:, mhh, pr, q, :] = cmT[sc, :, mhh, :]
    cntp = np.ascontiguousarray(
        cntp.reshape(128, 2 * NPR * M)).astype(np.float16)

    xpk = pack(np.ascontiguousarray(x.T), KH).astype(np.float16)

    in_maps = []
    for i in range(N_CORES):
        h0 = i * H_LOC
        c0 = i * OUT_C
        cols = np.concatenate([
            w_comb[:, (h0 + 0) * 192:(h0 + 0) * 192 + 128],
            w_comb[:, (h0 + 1) * 192:(h0 + 1) * 192 + 128],
            w_comb[:, (h0 + 0) * 192 + 128:(h0 + 1) * 192],
            w_comb[:, (h0 + 1) * 192 + 128:(h0 + 2) * 192],
        ], axis=1)                                           # [7168, 384]
        wcpk = pack(cols, KH).astype(np.float16)

        wqk_l = np.stack([
            pack(np.ascontiguousarray(W_qk[h].T), 4)
            for h in range(h0, h0 + H_LOC)], axis=1)
        wqk_l = np.ascontiguousarray(
            wqk_l.reshape(128, H_LOC * 4 * 128)).astype(np.float16)

        vw_l = np.stack([pack(vw[h], NSC)
                         for h in range(h0, h0 + H_LOC)], axis=1)
        vw_l = np.ascontiguousarray(
            vw_l.reshape(128, H_LOC * NSC * 128)).astype(np.float16)

        # gathered chunk n = rank n//2, local head n%2 = global head n
        order = [2 * k + h for k in range(N_CORES) for h in range(H_LOC)]
        wop_r = W_oproj.reshape(16, 128, HID)[order][:, :, c0:c0 + OUT_C]
        wop_l = pack(wop_r.reshape(16 * 128, OUT_C), 16).astype(np.float16)

        in_maps.append({
            "xp": xpk,
            "wcp": wcpk,
            "wqk2": wqk_l,
            "kvp": kvp,
            "kpe2d": kpe2,
            "cntp": cntp,
            "vwp": vw_l,
            "wopp": wop_l,
        })
    return in_maps


_prog_cache = {}


def kernel(x, W_cqkv, W_uq, W_qk, kv_cache, W_o1, W_oproj, indices):
    if "nc" not in _prog_cache:
        _prog_cache["nc"] = build_program()
    nc = _prog_cache["nc"]
    in_maps = prep_inputs(x, W_cqkv, W_uq, W_qk, kv_cache, W_o1, W_oproj,
                          indices)
    trace = bool(int(os.environ.get("KERNEL_TRACE", "0")))
    res = run_bass_kernel_spmd(nc, in_maps, list(range(N_CORES)),
                               trace=trace)
    _prog_cache["last_result"] = res
    out = np.empty((M, HID), np.float32)
    for i in range(N_CORES):
        out[:, i * OUT_C:(i + 1) * OUT_C] = res.results[i]["outT"].T
    return out


# revision 24
# speedup vs baseline: 1.1497x; 1.0475x over previous
"""DeepSeek MLA prefill (absorbed) on 8 Trainium2 NeuronCores — v4.

Sharding: tensor-parallel over heads (2 of 16 per core). Host-side
algebraic folds remove both the QKV-compression collective and most of
the attention FLOPs:
  - W_comb = W_cqkv[:, qc] @ W_uq   -> q computed straight from x.
  - k_abs = kv_c @ W_qk[h].T on device (192-dim score contraction).
  - VW[h] = V @ W_o1[h] on host     -> value+O-bmm in one matmul.
Top-k selection = dense count-weighted softmax. All f16, f32 PSUM.

Phases: q-GEMM streams x/W_comb first (overlapping the collectives'
init window), then k_abs, then attention twice over 256-token halves
so half A's o2 AllGather rides under half B's compute and the
O-projection pipelines under half B's AllGather. Inside a half,
sc-chunks run in pairs three-deep pipelined: scores (PE) -> exp (ACT,
one op per pair) -> xcnt (DVE) -> Z + value (PE). Z for both heads
packs into one PSUM bank (single start per bank; has_written handles
the second group).
"""

import os
import sys

sys.path.insert(0, "/opt/trn_rl_repo")

import numpy as np

import concourse.bass as bass
import concourse.tile as tile
from concourse import bacc, mybir
from concourse.bass_utils import run_bass_kernel_spmd

F32 = mybir.dt.float32
F16 = mybir.dt.float16

N_CORES = 8
M = 512
MH = M // 2
HID = 7168
D_KV_C, D_Q_C, D_R, D_Q = 512, 1536, 64, 128
H_LOC = 2
D_ATT = D_KV_C + D_R
S_KV = 4096
OUT_C = HID // N_CORES          # 896
KH = HID // 128                 # 56
NSC = S_KV // 128               # 32
NPR = NSC // 2                  # 16 sc-pairs
SM_SCALE = 1.0 / float(np.sqrt(np.float32(D_ATT)))
N_WARM = 48
LAG = 2


def build_program():
    nc = bacc.Bacc("TRN2", target_bir_lowering=False, debug=False,
                   num_devices=N_CORES)

    xp = nc.dram_tensor("xp", [128, KH * M], F16, kind="ExternalInput")
    wcp = nc.dram_tensor("wcp", [128, KH * 384], F16, kind="ExternalInput")
    wqk2 = nc.dram_tensor("wqk2", [128, H_LOC * 4 * 128], F16,
                          kind="ExternalInput")
    kvp = nc.dram_tensor("kvp", [128, 4 * S_KV], F16, kind="ExternalInput")
    kpe2d = nc.dram_tensor("kpe2d", [128, S_KV], F16, kind="ExternalInput")
    cntp = nc.dram_tensor("cntp", [128, 2 * NPR * M], F16,
                          kind="ExternalInput")
    vwp = nc.dram_tensor("vwp", [128, H_LOC * NSC * 128], F16,
                         kind="ExternalInput")
    wopp = nc.dram_tensor("wopp", [128, 16 * OUT_C], F16,
                          kind="ExternalInput")
    outT = nc.dram_tensor("outT", [OUT_C, M], F32, kind="ExternalOutput")

    rg = [list(range(N_CORES))]

    with tile.TileContext(nc) as tc, \
            nc.allow_low_precision(reason="f16 matmul pipeline, f32 accum"):
        with tc.tile_pool(name="dram", bufs=1, space="DRAM") as dram:
            o2l = [dram.tile([H_LOC * 128, MH], F16, name=f"o2l{mh}")
                   for mh in range(2)]
            o2a = [dram.tile([H_LOC * 128 * N_CORES, MH], F16,
                             name=f"o2a{mh}", addr_space="Shared")
                   for mh in range(2)]

            per_cm = tc.tile_pool(name="per", bufs=1)
            per = per_cm.__enter__()
            kabs = [per.tile([128, S_KV], F16, name=f"kabs{h}")
                    for h in range(H_LOC)]
            kpe_sb = per.tile([128, S_KV], F16, name="kpe")
            qn = [per.tile([128, M], F16, name=f"qn{h}")
                  for h in range(H_LOC)]
            qpe2 = per.tile([128, M], F16, name="qpe2")
            vw_sb = per.tile([128, H_LOC * NSC * 128], F16, name="vw")
            wop_sb = per.tile([128, 16 * OUT_C], F16, name="wop")
            ones_col = per.tile([128, 1], F16, name="ones_col")
            ones_colf = per.tile([128, 1], F32, name="ones_colf")
            ones_row = per.tile([1, 128], F32, name="ones_row")
            nc.vector.memset(ones_col[:], 1.0)
            nc.vector.memset(ones_colf[:], 1.0)
            nc.vector.memset(ones_row[:], 1.0)

            s1_cm = tc.tile_pool(name="s1", bufs=1)
            s1 = s1_cm.__enter__()
            warm = s1.tile([128, 64], F32, name="warm")
            nc.vector.memset(warm[:], 0.0)
            kv_sb = s1.tile([128, 4 * S_KV], F16, name="kv")
            wqk_sb = s1.tile([128, H_LOC * 4 * 128], F16, name="wqk")

            # ---------------- stage 1+2 fused: q = x @ W_comb ------------
            # (first so its DMA stream overlaps the collectives' init)
            with (
                tc.tile_pool(name="s2x", bufs=3) as s2x,
                tc.tile_pool(name="s2w", bufs=3) as s2w,
                tc.tile_pool(name="psq", bufs=1, space="PSUM") as psq,
                tc.tile_pool(name="psw", bufs=1, space="PSUM") as psw,
            ):
                wps = psw.tile([1, 64], F32, name="wps", tag="wps")
                for i in range(N_WARM):
                    nc.tensor.matmul(wps[:], warm[:, 0:1], warm[:],
                                     start=(i == 0), stop=(i == N_WARM - 1),
                                     skip_group_check=True)
                pq = [psq.tile([128, M], F32, name=f"q{j}", tag=f"q{j}")
                      for j in range(3)]
                NG = KH // 8   # 7 supergroups of 8 hid-chunks
                sg = []
                for g in range(NG):
                    wt = s2w.tile([128, 8 * 384], F16, name="wt", tag="wt")
                    nc.sync.dma_start(
                        wt[:], wcp[:, g * 8 * 384:(g + 1) * 8 * 384])
                    xa = s2x.tile([128, 2048], F16, name="xa", tag="xa")
                    nc.sync.dma_start(
                        xa[:], xp[:, (8 * g) * M:(8 * g + 4) * M])
                    xb = s2x.tile([128, 2048], F16, name="xb", tag="xb")
                    nc.sync.dma_start(
                        xb[:], xp[:, (8 * g + 4) * M:(8 * g + 8) * M])
                    if 1 <= g <= 4:
                        # spread k_abs input loads between supergroups
                        pcs = [(0, 0), (1, 0), (2, 0), (3, 0),
                               (0, 1), (1, 1), (2, 1), (3, 1)]
                        for c, half in pcs[(g - 1) * 2:(g - 1) * 2 + 2]:
                            nc.sync.dma_start(
                                kv_sb[:, c * S_KV + half * 2048:
                                      c * S_KV + (half + 1) * 2048],
                                kvp[:, c * S_KV + half * 2048:
                                    c * S_KV + (half + 1) * 2048])
                    if g == 5:
                        nc.sync.dma_start(wqk_sb[:], wqk2[:])
                        nc.sync.dma_start(kpe_sb[:, 0:2048],
                                          kpe2d[:, 0:2048])
                        nc.sync.dma_start(kpe_sb[:, 2048:4096],
                                          kpe2d[:, 2048:4096])
                    for kk in range(8):
                        xt = xa if kk < 4 else xb
                        rhs = xt[:, (kk % 4) * M:(kk % 4 + 1) * M]
                        for j in range(3):
                            nc.tensor.matmul(
                                pq[j][:],
                                wt[:, kk * 384 + j * 128:
                                   kk * 384 + (j + 1) * 128],
                                rhs,
                                start=(g == 0 and kk == 0),
                                stop=(g == NG - 1 and kk == 7),
                                skip_group_check=True)
                nc.vector.tensor_copy(qn[0][:], pq[0][:])
                nc.scalar.copy(qn[1][:], pq[1][:])
                nc.vector.tensor_copy(qpe2[:], pq[2][:])

            # ---------------- k_abs --------------------------------------
            with tc.tile_pool(name="psk", bufs=2, space="PSUM") as psk:
                nc.sync.dma_start(vw_sb[:, 0:S_KV], vwp[:, 0:S_KV])
                nc.sync.dma_start(vw_sb[:, S_KV:2 * S_KV],
                                  vwp[:, S_KV:2 * S_KV])
                for w8 in range(8):
                    nc.sync.dma_start(
                        wop_sb[:, w8 * 2 * OUT_C:(w8 + 1) * 2 * OUT_C],
                        wopp[:, w8 * 2 * OUT_C:(w8 + 1) * 2 * OUT_C])
                for pc in range(8):
                    for h in range(H_LOC):
                        kps = psk.tile([128, 512], F32, name="kps",
                                       tag="kps")
                        for c in range(4):
                            nc.tensor.matmul(
                                kps[:],
                                wqk_sb[:, (h * 4 + c) * 128:
                                       (h * 4 + c + 1) * 128],
                                kv_sb[:, c * S_KV + pc * 512:
                                      c * S_KV + (pc + 1) * 512],
                                start=(c == 0), stop=(c == 3))
                        if h == 0:
                            nc.vector.tensor_copy(
                                kabs[h][:, pc * 512:(pc + 1) * 512], kps[:])
                        else:
                            nc.scalar.copy(
                                kabs[h][:, pc * 512:(pc + 1) * 512], kps[:])
            s1_cm.__exit__(None, None, None)

            # ---------------- attention over m-halves --------------------
            for mh in range(2):
                m0 = mh * MH
                with (
                    tc.tile_pool(name=f"cnt{mh}", bufs=4) as cnts,
                    tc.tile_pool(name=f"exs{mh}", bufs=4) as exs,
                    tc.tile_pool(name=f"pts{mh}", bufs=4) as pts,
                    tc.tile_pool(name=f"psS{mh}", bufs=LAG,
                                 space="PSUM") as psS,
                    tc.tile_pool(name=f"psZ{mh}", bufs=1,
                                 space="PSUM") as psZ,
                ):
                    o2p = psZ.tile([128, 2 * MH], F32, name="o2p")
                    z_ps = [psZ.tile([1, M], F32, name=f"zp{h}")
                            for h in range(H_LOC)]
                    zacc = exs.tile([128, 2 * MH], F32, name="zacc")
                    nc.vector.memset(zacc[:], 0.0)
                    cnt_t = {}

                    def cnt_load(pr2):
                        t = cnts.tile([128, 2 * M], F16, name="cc",
                                      tag="cc")
                        nc.sync.dma_start(
                            t[:], cntp[:, (mh * NPR + 2 * pr2) * M:
                                       (mh * NPR + 2 * pr2 + 2) * M])
                        cnt_t[2 * pr2] = t[:, 0:M]
                        cnt_t[2 * pr2 + 1] = t[:, M:2 * M]

                    cnt_load(0)
                    cnt_load(1)

                    pend = {}

                    def emit_scores(pr):
                        ps = psS.tile([128, 4 * MH], F32, name="ss",
                                      tag="ss")
                        # one start per bank (h0 -> bank A, h1 -> bank B)
                        for q in range(2):
                            sc = 2 * pr + q
                            for h in range(H_LOC):
                                nc.tensor.matmul(
                                    ps[:, (h * 2 + q) * MH:
                                       (h * 2 + q + 1) * MH],
                                    kabs[h][:, sc * 128:(sc + 1) * 128],
                                    qn[h][:, m0:m0 + MH],
                                    start=(q == 0), stop=False,
                                    skip_group_check=True)
                        for q in range(2):
                            sc = 2 * pr + q
                            for h in range(H_LOC):
                                b = h * 64
                                nc.tensor.matmul(
                                    ps[:, (h * 2 + q) * MH:
                                       (h * 2 + q + 1) * MH],
                                    kpe_sb[b:b + 64,
                                           sc * 128:(sc + 1) * 128],
                                    qpe2[b:b + 64, m0:m0 + MH],
                                    start=False, stop=True,
                                    skip_group_check=True,
                                    tile_position=(b, 0))
                        pend[pr] = ps

                    def consume(pr):
                        ps = pend.pop(pr)
                        ex = exs.tile([128, 4 * MH], F16, name="ex",
                                      tag="ex")
                        nc.scalar.activation(
                            ex[:], ps[:],
                            mybir.ActivationFunctionType.Exp,
                            scale=SM_SCALE)
                        for h in range(H_LOC):
                            pt = pts.tile([128, 2 * MH], F16, name="pt",
                                          tag="pt")
                            nc.vector.tensor_mul(
                                pt[:], ex[:, h * 2 * MH:(h + 1) * 2 * MH],
                                cnt_t[pr])
                            if h == 0:
                                nc.tensor.matmul(
                                    z_ps[h][:], ones_col[:], pt[:],
                                    start=(pr == 0), stop=(pr == NPR - 1),
                                    skip_group_check=True)
                            else:
                                nc.vector.tensor_add(zacc[:], zacc[:],
                                                     pt[:])
                            for q in range(2):
                                sc = 2 * pr + q
                                nc.tensor.matmul(
                                    o2p[:, h * MH:(h + 1) * MH],
                                    vw_sb[:, (h * NSC + sc) * 128:
                                          (h * NSC + sc + 1) * 128],
                                    pt[:, q * MH:(q + 1) * MH],
                                    start=(pr == 0 and q == 0 and h == 0),
                                    stop=(pr == NPR - 1 and q == 1),
                                    skip_group_check=True)

                    for pr in range(NPR):
                        if pr % 2 == 0 and pr // 2 + 2 < NPR // 2:
                            cnt_load(pr // 2 + 2)
                        emit_scores(pr)
                        if pr >= LAG:
                            consume(pr - LAG)
                    for pr in range(NPR - LAG, NPR):
                        consume(pr)

                    # Z -> 1/Z broadcast -> normalize -> AllGather
                    nc.tensor.matmul(z_ps[1][:], ones_colf[:], zacc[:],
                                     start=True, stop=True,
                                     skip_group_check=True)
                    zs_l = []
                    for h in range(H_LOC):
                        zsb = exs.tile([1, M], F32, name="zsb", tag="zsb")
                        nc.scalar.copy(zsb[:], z_ps[h][:])
                        zs = exs.tile([1, MH], F32, name="zs", tag="zs")
                        nc.vector.tensor_add(zs[:], zsb[0:1, 0:MH],
                                             zsb[0:1, MH:2 * MH])
                        zs_l.append(zs)
                    zbt = psS.tile([128, 4 * MH], F32, name="ss", tag="ss")
                    nc.tensor.matmul(zbt[:, 0:MH], ones_row[:],
                                     zs_l[0][:], start=True, stop=True,
                                     skip_group_check=True)
                    nc.tensor.matmul(zbt[:, MH:2 * MH], ones_row[:],
                                     zs_l[1][:], start=False, stop=True,
                                     skip_group_check=True)
                    zbsb = exs.tile([128, 2 * MH], F32, name="zbsb",
                                    tag="zbsb")
                    nc.scalar.copy(zbsb[:], zbt[:, 0:2 * MH])
                    rzb = exs.tile([128, 2 * MH], F32, name="rzb",
                                   tag="rzb")
                    nc.vector.reciprocal_approx_fast(rzb[:], zbsb[:])
                    for h in range(H_LOC):
                        o2s = pts.tile([128, MH], F16, name=f"o2s{h}")
                        nc.vector.tensor_mul(
                            o2s[:], o2p[:, h * MH:(h + 1) * MH],
                            rzb[:, h * MH:(h + 1) * MH])
                        nc.sync.dma_start(
                            o2l[mh][h * 128:(h + 1) * 128, :], o2s[:])
                    nc.gpsimd.collective_compute(
                        "AllGather", mybir.AluOpType.bypass,
                        replica_groups=rg,
                        ins=[o2l[mh].opt()], outs=[o2a[mh].opt()])
                    for f in range(10 if mh == 0 else 4):
                        nc.tensor.matmul(
                            zbt[:, 2 * MH:4 * MH],
                            vw_sb[:, 0:128], vw_sb[:, 0:M],
                            start=(f == 0), stop=True,
                            skip_group_check=True)

            # ---------------- O-projection (per half, p-outer) -----------
            for mh in range(2):
                with (
                    tc.tile_pool(name=f"s6{mh}", bufs=1) as s6,
                    tc.tile_pool(name=f"ps6{mh}", bufs=2,
                                 space="PSUM") as ps6,
                    tc.tile_pool(name=f"s6o{mh}", bufs=4) as s6o,
                ):
                    o2t = []
                    for n in range(16):
                        ok = s6.tile([128, MH], F16, name=f"o2t{n}")
                        nc.sync.dma_start(
                            ok[:], o2a[mh][n * 128:(n + 1) * 128, :])
                        o2t.append(ok)
                    fill = ps6.tile([128, 512], F32, name="pb", tag="pb")
                    for f in range(40 if mh == 0 else 14):
                        nc.tensor.matmul(
                            fill[:, 0:M], vw_sb[:, 0:128], vw_sb[:, 0:M],
                            start=(f == 0), stop=True,
                            skip_group_check=True)
                    for p in range(OUT_C // 128):
                        pb = ps6.tile([128, 512], F32, name="pb", tag="pb")
                        for n, ok in enumerate(o2t):
                            nc.tensor.matmul(
                                pb[:, 0:MH],
                                wop_sb[:, n * OUT_C + p * 128:
                                       n * OUT_C + (p + 1) * 128],
                                ok[:], start=(n == 0), stop=(n == 15),
                                skip_group_check=True)
                        ob = s6o.tile([128, MH], F32, name="outb",
                                      tag="outb")
                        if p % 2 == 0:
                            nc.vector.tensor_copy(ob[:], pb[:, 0:MH])
                        else:
                            nc.scalar.copy(ob[:], pb[:, 0:MH])
                        nc.sync.dma_start(
                            outT[p * 128:(p + 1) * 128,
                                 mh * MH:(mh + 1) * MH], ob[:])

            per_cm.__exit__(None, None, None)

    nc.compile()
    return nc


def prep_inputs(x, W_cqkv, W_uq, W_qk, kv_cache, W_o1, W_oproj, indices):
    x = np.asarray(x, np.float32)
    W_cqkv = np.asarray(W_cqkv, np.float32)
    W_uq = np.asarray(W_uq, np.float32)
    W_qk = np.asarray(W_qk, np.float32)
    kv_cache = np.asarray(kv_cache, np.float32)
    W_o1 = np.asarray(W_o1, np.float32)
    W_oproj = np.asarray(W_oproj, np.float32)
    indices = np.asarray(indices)

    # host-side algebraic folds (f32)
    w_comb = W_cqkv[:, D_KV_C:D_KV_C + D_Q_C] @ W_uq        # [7168, 3072]
    vw = np.einsum("sc,hcv->hsv", kv_cache[:, :D_KV_C], W_o1)  # [16,4096,128]

    def pack(a, nchunk):
        # [nchunk*128, F] -> [128, nchunk*F]
        f = a.shape[1]
        return np.ascontiguousarray(
            a.reshape(nchunk, 128, f).transpose(1, 0, 2).reshape(
                128, nchunk * f))

    kvT = kv_cache.T                                         # [576, 4096]
    kvp = pack(kvT[:D_KV_C], 4).astype(np.float16)
    kpe2 = np.concatenate([kvT[D_KV_C:], kvT[D_KV_C:]], 0).astype(np.float16)

    cm = np.zeros((M, S_KV), np.float32)
    np.add.at(cm, (np.arange(M)[:, None], indices), 1.0)
    # cnt_pack[p, mh, pr, q, m256] = cm[mh*256+m, 128*(2pr+q)+p]
    cmT = np.ascontiguousarray(cm.T).reshape(NSC, 128, 2, MH)
    cntp = np.empty((128, 2, NPR, 2, MH), np.float32)
    for pr in range(NPR):
        for q in range(2):
            sc = 2 * pr + q
            for mhh in range(2):
                cntp[:, mhh, pr, q, :] = cmT[sc, :, mhh, :]
    cntp = np.ascontiguousarray(
        cntp.reshape(128, 2 * NPR * M)).astype(np.float16)

    xpk = pack(np.ascontiguousarray(x.T), KH).astype(np.float16)

    in_maps = []
    for i in range(N_CORES):
        h0 = i * H_LOC
        c0 = i * OUT_C
        cols = np.concatenate([
            w_comb[:, (h0 + 0) * 192:(h0 + 0) * 192 + 128],
            w_comb[:, (h0 + 1) * 192:(h0 + 1) * 192 + 128],
            w_comb[:, (h0 + 0) * 192 + 128:(h0 + 1) * 192],
            w_comb[:, (h0 + 1) * 192 + 128:(h0 + 2) * 192],
        ], axis=1)                                           # [7168, 384]
        wcpk = pack(cols, KH).astype(np.float16)

        wqk_l = np.stack([
            pack(np.ascontiguousarray(W_qk[h].T), 4)
            for h in range(h0, h0 + H_LOC)], axis=1)
        wqk_l = np.ascontiguousarray(
            wqk_l.reshape(128, H_LOC * 4 * 128)).astype(np.float16)

        vw_l = np.stack([pack(vw[h], NSC)
                         for h in range(h0, h0 + H_LOC)], axis=1)
        vw_l = np.ascontiguousarray(
            vw_l.reshape(128, H_LOC * NSC * 128)).astype(np.float16)

        # gathered chunk n = rank n//2, local head n%2 = global head n
        order = [2 * k + h for k in range(N_CORES) for h in range(H_LOC)]
        wop_r = W_oproj.reshape(16, 128, HID)[order][:, :, c0:c0 + OUT_C]
        wop_l = pack(wop_r.reshape(16 * 128, OUT_C), 16).astype(np.float16)

        in_maps.append({
            "xp": xpk,
            "wcp": wcpk,
            "wqk2": wqk_l,
            "kvp": kvp,
            "kpe2d": kpe2,
            "cntp": cntp,
            "vwp": vw_l,
            "wopp": wop_l,
        })
    return in_maps


_prog_cache = {}


def kernel(x, W_cqkv, W_uq, W_qk, kv_cache, W_o1, W_oproj, indices):
    if "nc" not in _prog_cache:
        _prog_cache["nc"] = build_program()
    nc = _prog_cache["nc"]
    in_maps = prep_inputs(x, W_cqkv, W_uq, W_qk, kv_cache, W_o1, W_oproj,
                          indices)
    trace = bool(int(os.environ.get("KERNEL_TRACE", "0")))
    res = run_bass_kernel_spmd(nc, in_maps, list(range(N_CORES)),
                               trace=trace)
    _prog_cache["last_result"] = res
    out = np.empty((M, HID), np.float32)
    for i in range(N_CORES):
        out[:, i * OUT_C:(i + 1) * OUT_C] = res.results[i]["outT"].T
    return out
